# revision 1
# baseline (speedup 1.0000x reference)
"""MoE layer (E=8 experts, top-2 routing, D=1024, hidden 4096, GELU) on 8
Trainium2 NeuronCores.

Strategy: expert parallelism. The router (gate matmul + top-k + softmax) is
computed on the host with the exact same jax calls as the reference (so the
routing decisions match bit-for-bit), tokens are gathered per expert and
dispatched to one core per expert. Each core runs the expert MLP
  y = gelu(x @ w1[e]) @ w2[e]
for its (capacity-padded) token set in float32r (TF32-like full-speed PE
mode, ~2e-4 relative error). The hidden dimension is processed in four
passes of 1024 whose weight SBUF slots are ping-ponged (pass p+1's weights
stream in under pass p's compute), with partial outputs accumulated across
passes through a DRAM scratch tensor; token blocks are sized so every
matmul's moving dim is >=256 (full fp32r rate) and covers the ~190ns
stationary weight load. The host then applies the gate coefficients and
scatter-adds the two expert outputs per token in expert-index order,
matching the reference accumulation order.
"""

import numpy as np

D = 1024        # token dim (8 chunks of 128)
E = 8           # experts == cores
HH = 4096       # hidden width (2*H)
NQ = 4          # hidden-dim passes (quarters, ping-ponged weight slots)
HQ = HH // NQ   # per-pass hidden width (1024)
NK = D // 128    # k-chunks (8)
NH = HQ // 128   # hh-chunks per pass (8)
ND = D // 128    # output d-chunks (8)
TB = 512        # token block (psum bank width in fp32)

_BUILD_CACHE = {}
_TRACE = False      # test-only: capture an NTFF profile of the run
_LAST_RES = None    # test-only: last BassKernelResults


def _block_sizes(cap):
    """Token-block sizes for a given capacity. Matmuls with a 512-wide
    moving operand issue at 1 cycle/row; narrower ones are bound by the
    ~190ns stationary weight load (flat for widths 256..~420). So prefer
    512-wide blocks and make the remainder blocks <= ~420 wide."""
    if cap <= TB:
        return [max(256, -(-cap // 4) * 4)]
    nblk = -(-cap // TB)
    sizes = None
    for n512 in range(nblk + 1):
        m = nblk - n512
        if m == 0:
            if TB * n512 >= cap:
                sizes = [TB] * n512
                break
            continue
        small = -(-(cap - TB * n512) // (4 * m)) * 4
        if 256 <= small <= 420:
            sizes = [TB] * n512 + [small] * m
            break
    if sizes is None:
        sizes = [TB] * nblk
    # shrink one full block to exactly the needed coverage (multiple of 4,
    # >= 256): fewer padded tokens and a slightly cheaper matmul spacing
    excess = (sum(sizes) - cap) // 4 * 4
    if excess > 0 and sizes[0] == TB and sizes[0] - excess >= 256:
        sizes[0] -= excess
        sizes.sort(reverse=True)
    return sizes


def _build(cap, act="gelu"):
    """Build + compile the per-core Bass program for capacity `cap`.
    Returns (compiled Bass object, padded capacity)."""
    cap = sum(_block_sizes(cap))
    key = (cap, act)
    if key in _BUILD_CACHE:
        return _BUILD_CACHE[key]

    import concourse.mybir as mybir
    import concourse.tile as tile
    from concourse import bacc

    f32 = mybir.dt.float32
    f32r = mybir.dt.float32r
    GELU = (mybir.ActivationFunctionType.Gelu if act == "gelu"
            else mybir.ActivationFunctionType.Tanh)

    nc = bacc.Bacc("TRN2", target_bir_lowering=False, debug=False,
                   num_devices=E)

    xT = nc.dram_tensor("xT", [NK, 128, cap], f32r, kind="ExternalInput")
    w1 = nc.dram_tensor("w1", [NQ, NK, 128, HQ], f32r, kind="ExternalInput")
    w2 = nc.dram_tensor("w2", [NQ, NH, 128, D], f32r, kind="ExternalInput")
    yT = nc.dram_tensor("yT", [ND, 128, cap], f32, kind="ExternalOutput")

    sizes = _block_sizes(cap)
    blocks = []
    t0 = 0
    for tb in sizes:
        blocks.append((t0, tb))
        t0 += tb

    with tile.TileContext(nc) as tc:
        with (
            tc.tile_pool(name="w1p", bufs=2) as w1p,
            tc.tile_pool(name="w2p", bufs=2) as w2p,
            tc.tile_pool(name="xp", bufs=2) as xp,
            tc.tile_pool(name="hp", bufs=1) as hp,
            tc.tile_pool(name="yp", bufs=4) as ypool,
            tc.tile_pool(name="yin", bufs=3) as yinp,
            tc.tile_pool(name="dram", bufs=1, space="DRAM") as dram,
            tc.tile_pool(name="ps1", bufs=4, space="PSUM") as ps1,
            tc.tile_pool(name="ps2", bufs=4, space="PSUM") as ps2,
        ):
            ypart = dram.tile([ND, 128, cap], f32)

            for p in range(NQ):
                # weight quarter for this pass; bufs=2 tags ping-pong the
                # slots so pass p+1's loads overlap pass p's compute
                w1sb = [
                    w1p.tile([128, HQ], f32r, name=f"w1_{p}_{k}",
                             tag=f"w1_{k}")
                    for k in range(NK)
                ]
                w2sb = [
                    w2p.tile([128, D], f32r, name=f"w2_{p}_{h}",
                             tag=f"w2_{h}")
                    for h in range(NH)
                ]
                if p > 0:
                    for k in range(NK):
                        nc.sync.dma_start(w1sb[k][:], w1.ap()[p][k])
                    for h in range(NH):
                        nc.sync.dma_start(w2sb[h][:], w2.ap()[p][h])

                # boustrophedon: alternate passes walk the blocks in reverse
                # so the boundary block's x tiles are reused without a reload
                order = blocks if p % 2 == 0 else blocks[::-1]
                for gi, (t0, tb) in enumerate(order):
                    g = blocks.index((t0, tb))
                    if p > 0 and gi == 0:
                        xt = xt_prev  # same tokens, still resident
                    else:
                        xt = [
                            xp.tile([128, TB], f32r, name=f"x_{p}_{g}_{k}",
                                    tag=f"x_{k}")
                            for k in range(NK)
                        ]
                        for k in range(NK):
                            nc.sync.dma_start(xt[k][:, :tb],
                                              xT.ap()[k][:, t0:t0 + tb])
                    xt_prev = xt
                    if p == 0 and gi == 0:
                        # first pass: w1 quarter + first x block gate the
                        # first matmul, so they get the DMA queues first
                        for k in range(NK):
                            nc.sync.dma_start(w1sb[k][:], w1.ap()[p][k])

                    # GEMM1 + GELU: h[n] = gelu(w1[:, n].T @ x)
                    ht = [
                        hp.tile([128, TB], f32r, name=f"h_{p}_{g}_{n}",
                                tag=f"h_{n}")
                        for n in range(NH)
                    ]
                    for n in range(NH):
                        acc = ps1.tile([128, tb], f32,
                                       name=f"ps1_{p}_{g}_{n}", tag="ps1")
                        for k in range(NK):
                            nc.tensor.matmul(
                                acc[:, :tb],
                                w1sb[k][:, n * 128:(n + 1) * 128],
                                xt[k][:, :tb],
                                start=(k == 0),
                                stop=(k == NK - 1),
                            )
                        nc.scalar.activation(ht[n][:, :tb], acc[:, :tb],
                                             GELU)

                    if p == 0 and gi == 0:
                        # w2 is first needed here, ~55us after kernel start;
                        # emitting its loads after GEMM1 keeps them out of
                        # the critical head DMA window
                        for h in range(NH):
                            nc.sync.dma_start(w2sb[h][:], w2.ap()[p][h])

                    # GEMM2: y[d] += w2[:, d].T @ h  (accumulated over passes
                    # through a DRAM scratch tensor)
                    for d in range(ND):
                        acc2 = ps2.tile([128, tb], f32,
                                        name=f"ps2_{p}_{g}_{d}", tag="ps2")
                        for h in range(NH):
                            nc.tensor.matmul(
                                acc2[:, :tb],
                                w2sb[h][:, d * 128:(d + 1) * 128],
                                ht[h][:, :tb],
                                start=(h == 0),
                                stop=(h == NH - 1),
                            )
                        yt = ypool.tile([128, TB], f32,
                                        name=f"y_{p}_{g}_{d}", tag="y")
                        if p == 0:
                            nc.vector.tensor_copy(yt[:, :tb], acc2[:, :tb])
                        else:
                            yprev = yinp.tile([128, TB], f32,
                                              name=f"yi_{p}_{g}_{d}",
                                              tag="yi")
                            nc.sync.dma_start(yprev[:, :tb],
                                              ypart[d][:, t0:t0 + tb])
                            nc.vector.tensor_add(yt[:, :tb], acc2[:, :tb],
                                                 yprev[:, :tb])
                        if p == NQ - 1:
                            nc.sync.dma_start(yT.ap()[d][:, t0:t0 + tb],
                                              yt[:, :tb])
                        else:
                            nc.sync.dma_start(ypart[d][:, t0:t0 + tb],
                                              yt[:, :tb])

    nc.compile()
    _BUILD_CACHE[key] = (nc, cap)
    return nc, cap


def _route(x, gate_w):
    """Mirror the reference router with the exact same jax calls on the
    process-default backend, so the (discrete) top-k decisions match the
    reference bit-for-bit when the grader runs both in one environment.
    Falls back to CPU if the default backend fails."""
    import jax
    import jax.numpy as jnp

    def run():
        logits = jnp.einsum("btd,de->bte", jnp.asarray(x),
                            jnp.asarray(gate_w))
        scores, indices = jax.lax.top_k(logits, 2)
        gates = jax.nn.softmax(scores, axis=-1)
        return (np.asarray(indices).reshape(-1, 2),
                np.asarray(gates, dtype=np.float32).reshape(-1, 2))

    try:
        return run()
    except Exception:
        with jax.default_device(jax.devices("cpu")[0]):
            return run()


def kernel(x, gate_w, w1, w2):
    from concourse.bass_utils import run_bass_kernel_spmd

    x = np.asarray(x, dtype=np.float32)
    gate_w = np.asarray(gate_w, dtype=np.float32)
    w1 = np.asarray(w1, dtype=np.float32)
    w2 = np.asarray(w2, dtype=np.float32)

    B, T, _ = x.shape
    xf = x.reshape(-1, D)
    ntok = xf.shape[0]

    indices, gates = _route(x, gate_w)

    rows = []
    coefs = []
    for e in range(E):
        sel0 = indices[:, 0] == e
        sel1 = indices[:, 1] == e
        r = np.nonzero(sel0 | sel1)[0]
        c = np.where(sel0[r], gates[r, 0], gates[r, 1])
        rows.append(r)
        coefs.append(c.astype(np.float32))

    max_cnt = max(len(r) for r in rows)
    nc, cap = _build(max(256, max_cnt))

    in_maps = []
    for e in range(E):
        r = rows[e]
        xe = np.zeros((D, cap), dtype=np.float32)
        xe[:, :len(r)] = xf[r].T
        in_maps.append({
            "xT": np.ascontiguousarray(xe.reshape(NK, 128, cap)),
            "w1": np.ascontiguousarray(
                w1[e].reshape(NK, 128, NQ, HQ).transpose(2, 0, 1, 3)),
            "w2": np.ascontiguousarray(w2[e].reshape(NQ, NH, 128, D)),
        })

    res = run_bass_kernel_spmd(nc, in_maps, core_ids=list(range(E)),
                               trace=_TRACE)
    global _LAST_RES
    _LAST_RES = res

    out = np.zeros((ntok, D), dtype=np.float32)
    for e in range(E):
        r = rows[e]
        ye = res.results[e]["yT"].reshape(D, cap)
        out[r] += coefs[e][:, None] * ye[:, :len(r)].T
    return out.reshape(B, T, D)



# revision 2
# speedup vs baseline: 1.1000x; 1.1000x over previous
"""MoE layer (E=8 experts, top-2 routing, D=1024, hidden 4096, GELU) on 8
Trainium2 NeuronCores.

Strategy: expert parallelism, one expert per core. The router (gate matmul +
top-k + softmax) runs on the host with the exact jax calls of the reference,
tokens are gathered per expert and dispatched to one core each. Each core
runs the expert MLP
  y = gelu(x @ w1[e]) @ w2[e]
for its (capacity-padded) token set in bf16 (PE runs bf16 at the same
1 col/cycle as fp32r, but FWL halves the stationary-weight load so narrow
remainder blocks aren't LDWEIGHTS-bound, and weights shrink 2x).

Both weight matrices live fully resident in SBUF (16.8 MB bf16), so there
is no per-pass weight streaming and no DRAM scratch round-trip for partial
outputs: per 512-token block, GEMM1 produces the full 4096-wide hidden in
SBUF (bf16) and GEMM2 immediately contracts it back down, writing the final
f32 output once. Blocks are sized in [256, 512] so every matmul's moving
dim is >=256 (full rate). The host then applies the gate coefficients and
scatter-adds the two expert outputs per token.
"""

import numpy as np

D = 1024        # token dim
E = 8           # experts == cores
HH = 4096       # hidden width (2*H)
NK = D // 128   # contraction chunks for GEMM1 (8)
NH = HH // 128  # hidden chunks (32)
ND = D // 128   # output chunks (8)
TB = 512        # max token block (psum bank width in fp32)

_BUILD_CACHE = {}
_TRACE = False      # test-only: capture an NTFF profile of the run
_LAST_RES = None    # test-only: last BassKernelResults


def _block_sizes(cap):
    """Split cap into blocks of width in [256, 512] (multiples of 4) so no
    matmul is LDWEIGHTS/overhead-bound. Widths 512 preferred."""
    cap = max(256, -(-cap // 4) * 4)
    nfull, rem = divmod(cap, TB)
    if rem == 0:
        return [TB] * nfull
    if rem >= 256 or nfull == 0:
        return [TB] * nfull + [max(256, rem)]
    # split the remainder + one full block into two blocks in [256, 512]
    tot = TB + rem
    a = -(-tot // 8) * 4
    return [TB] * (nfull - 1) + [a, tot - a]


def _build(cap):
    """Build + compile the per-core Bass program for capacity `cap`."""
    sizes = _block_sizes(cap)
    cap = sum(sizes)
    if cap in _BUILD_CACHE:
        return _BUILD_CACHE[cap]

    import concourse.mybir as mybir
    import concourse.tile as tile
    from concourse import bacc

    f32 = mybir.dt.float32
    bf16 = mybir.dt.bfloat16
    GELU = mybir.ActivationFunctionType.Gelu

    nc = bacc.Bacc("TRN2", target_bir_lowering=False, debug=False,
                   num_devices=E)

    xT = nc.dram_tensor("xT", [NK, 128, cap], bf16, kind="ExternalInput")
    # w1t[h][p][k][m] = w1[k*128+p, h*128+m]
    w1 = nc.dram_tensor("w1", [NH, 128, NK * 128], bf16, kind="ExternalInput")
    # w2t[d][p][hk][m] = w2[hk*128+p, d*128+m]
    w2 = nc.dram_tensor("w2", [ND, 128, NH * 128], bf16, kind="ExternalInput")
    yT = nc.dram_tensor("yT", [ND, 128, cap], f32, kind="ExternalOutput")

    blocks = []
    t0 = 0
    for tb in sizes:
        blocks.append((t0, tb))
        t0 += tb

    with tile.TileContext(nc) as tc:
        with (
            tc.tile_pool(name="w1p", bufs=1) as w1p,
            tc.tile_pool(name="w2p", bufs=1) as w2p,
            tc.tile_pool(name="xp", bufs=2) as xp,
            tc.tile_pool(name="hp", bufs=1) as hp,
            tc.tile_pool(name="yp", bufs=4) as ypool,
            tc.tile_pool(name="ps1", bufs=3, space="PSUM") as ps1,
            tc.tile_pool(name="ps2", bufs=3, space="PSUM") as ps2,
        ):
            # resident weights
            w1sb = [w1p.tile([128, NK, 128], bf16, name=f"w1_{h}")
                    for h in range(NH)]
            w2sb = [w2p.tile([128, NH, 128], bf16, name=f"w2_{d}")
                    for d in range(ND)]

            def load_x(g, t0, tb):
                xt = [xp.tile([128, TB], bf16, name=f"x_{g}_{k}",
                              tag=f"x_{k}") for k in range(NK)]
                for k in range(NK):
                    nc.sync.dma_start(xt[k][:, :tb], xT.ap()[k][:, t0:t0 + tb])
                return xt

            # head: first GEMM1 chunk is gated by w1[0] + x block 0 only
            nc.sync.dma_start(w1sb[0][:], w1.ap()[0])
            xts = load_x(0, *blocks[0])
            for h in range(1, NH):
                nc.sync.dma_start(w1sb[h][:], w1.ap()[h])

            for g, (t0, tb) in enumerate(blocks):
                xt = xts
                if g + 1 < len(blocks):
                    xts = load_x(g + 1, *blocks[g + 1])

                # GEMM1 + GELU: h[:, n, :] = gelu(w1[n].T @ x), bf16
                ht = hp.tile([128, NH, TB], bf16, name=f"h_{g}", tag="h")
                for n in range(NH):
                    acc = ps1.tile([128, tb], f32, name=f"ps1_{g}_{n}",
                                   tag="ps1")
                    for k in range(NK):
                        nc.tensor.matmul(
                            acc[:, :tb],
                            w1sb[n][:, k, :],
                            xt[k][:, :tb],
                            start=(k == 0),
                            stop=(k == NK - 1),
                        )
                    nc.scalar.activation(ht[:, n, :tb], acc[:, :tb], GELU)

                if g == 0:
                    # w2 is first needed ~55us in; emitting its loads after
                    # GEMM1 keeps them out of the critical head DMA window
                    for d in range(ND):
                        nc.sync.dma_start(w2sb[d][:], w2.ap()[d])

                # GEMM2: y[d] = w2[d].T @ h
                for d in range(ND):
                    acc2 = ps2.tile([128, tb], f32, name=f"ps2_{g}_{d}",
                                    tag="ps2")
                    for hk in range(NH):
                        nc.tensor.matmul(
                            acc2[:, :tb],
                            w2sb[d][:, hk, :],
                            ht[:, hk, :tb],
                            start=(hk == 0),
                            stop=(hk == NH - 1),
                        )
                    yt = ypool.tile([128, TB], f32, name=f"y_{g}_{d}",
                                    tag="y")
                    nc.vector.tensor_copy(yt[:, :tb], acc2[:, :tb])
                    nc.sync.dma_start(yT.ap()[d][:, t0:t0 + tb], yt[:, :tb])

    nc.compile()
    _BUILD_CACHE[cap] = (nc, cap)
    return nc, cap


def _route(x, gate_w):
    """Mirror the reference router with the exact same jax calls on the
    process-default backend, so the (discrete) top-k decisions match the
    reference bit-for-bit. Falls back to CPU if the default backend fails."""
    import jax
    import jax.numpy as jnp

    def run():
        logits = jnp.einsum("btd,de->bte", jnp.asarray(x),
                            jnp.asarray(gate_w))
        scores, indices = jax.lax.top_k(logits, 2)
        gates = jax.nn.softmax(scores, axis=-1)
        return (np.asarray(indices).reshape(-1, 2),
                np.asarray(gates, dtype=np.float32).reshape(-1, 2))

    try:
        return run()
    except Exception:
        with jax.default_device(jax.devices("cpu")[0]):
            return run()


def kernel(x, gate_w, w1, w2):
    import ml_dtypes
    from concourse.bass_utils import run_bass_kernel_spmd

    bf16 = ml_dtypes.bfloat16

    x = np.asarray(x, dtype=np.float32)
    gate_w = np.asarray(gate_w, dtype=np.float32)
    w1 = np.asarray(w1, dtype=np.float32)
    w2 = np.asarray(w2, dtype=np.float32)

    B, T, _ = x.shape
    xf = x.reshape(-1, D)
    ntok = xf.shape[0]

    indices, gates = _route(x, gate_w)

    rows = []
    coefs = []
    for e in range(E):
        sel0 = indices[:, 0] == e
        sel1 = indices[:, 1] == e
        r = np.nonzero(sel0 | sel1)[0]
        c = np.where(sel0[r], gates[r, 0], gates[r, 1])
        rows.append(r)
        coefs.append(c.astype(np.float32))

    max_cnt = max(len(r) for r in rows)
    nc, cap = _build(max_cnt)

    in_maps = []
    for e in range(E):
        r = rows[e]
        xe = np.zeros((D, cap), dtype=np.float32)
        xe[:, :len(r)] = xf[r].T
        in_maps.append({
            "xT": np.ascontiguousarray(
                xe.reshape(NK, 128, cap).astype(bf16)),
            "w1": np.ascontiguousarray(
                w1[e].reshape(NK, 128, NH, 128).transpose(2, 1, 0, 3)
                .reshape(NH, 128, NK * 128).astype(bf16)),
            "w2": np.ascontiguousarray(
                w2[e].reshape(NH, 128, ND, 128).transpose(2, 1, 0, 3)
                .reshape(ND, 128, NH * 128).astype(bf16)),
        })

    res = run_bass_kernel_spmd(nc, in_maps, core_ids=list(range(E)),
                               trace=_TRACE)
    global _LAST_RES
    _LAST_RES = res

    out = np.zeros((ntok, D), dtype=np.float32)
    for e in range(E):
        r = rows[e]
        ye = res.results[e]["yT"].reshape(D, cap)
        out[r] += coefs[e][:, None] * ye[:, :len(r)].T
    return out.reshape(B, T, D)


# revision 3
# speedup vs baseline: 1.1681x; 1.0619x over previous
"""MoE layer (E=8 experts, top-2 routing, D=1024, hidden 4096, GELU) on 8
Trainium2 NeuronCores.

Strategy: expert parallelism, one expert per core. The router (gate matmul +
top-k + softmax) runs on the host with the exact jax calls of the reference,
tokens are gathered per expert and dispatched to one core each. Each core
runs the expert MLP
  y = gelu(x @ w1[e]) @ w2[e]
for its (capacity-padded) token set, mostly in bf16 (PE runs bf16 at the
same 1 col/cycle as fp32r, but FWL halves the stationary-weight load so
narrow remainder blocks aren't LDWEIGHTS-bound, and weights shrink 2x).
The first NH8=8 of GEMM2's 32 contraction chunks run as fp8e4 DoubleRow
matmuls (2 MACs/cell/cycle), which trims ~12% off GEMM2; the fp8
quantization error on a quarter of the contraction keeps the end-to-end
max relative error at ~1.7e-2 (vs 2e-2 budget; all-bf16 is 3.7e-3).
w2 is pre-scaled by 1024 (so its fp8 chunks stay out of the subnormal
range) and the output copy divides it back out.

Both weight matrices live fully resident in SBUF, so there is no per-pass
weight streaming and no DRAM scratch round-trip for partial outputs: per
512-token block, GEMM1 produces the full 4096-wide hidden in SBUF and GEMM2
immediately contracts it back down, writing the final f32 output once.
Blocks are sized in [256, 512] so every matmul's moving dim is >=256.
The host applies the gate coefficients and scatter-adds per token.
"""

import numpy as np

D = 1024        # token dim
E = 8           # experts == cores
HH = 4096       # hidden width (2*H)
NK = D // 128   # contraction chunks for GEMM1 (8)
NH = HH // 128  # hidden chunks (32)
NH8 = 8         # leading GEMM2 chunks computed in fp8e4 DoubleRow
W2S = 1024.0    # w2 pre-scale (divided out in the output copy)
ND = D // 128   # output chunks (8)
TB = 512        # max token block (psum bank width in fp32)

_BUILD_CACHE = {}
_TRACE = False      # test-only: capture an NTFF profile of the run
_LAST_RES = None    # test-only: last BassKernelResults


def _block_sizes(cap):
    """Split cap into blocks of width in [256, 512] (multiples of 4) so no
    matmul is LDWEIGHTS/overhead-bound. Widths 512 preferred."""
    cap = max(256, -(-cap // 4) * 4)
    nfull, rem = divmod(cap, TB)
    if rem == 0:
        return [TB] * nfull
    if rem >= 256 or nfull == 0:
        return [TB] * nfull + [max(256, rem)]
    # split the remainder + one full block into two blocks in [256, 512]
    tot = TB + rem
    a = -(-tot // 8) * 4
    return [TB] * (nfull - 1) + [a, tot - a]


def _build(cap):
    """Build + compile the per-core Bass program for capacity `cap`."""
    sizes = _block_sizes(cap)
    cap = sum(sizes)
    if cap in _BUILD_CACHE:
        return _BUILD_CACHE[cap]

    import concourse.mybir as mybir
    import concourse.tile as tile
    from concourse import bacc

    f32 = mybir.dt.float32
    bf16 = mybir.dt.bfloat16
    f8 = mybir.dt.float8e4
    DR = mybir.MatmulPerfMode.DoubleRow
    GELU = mybir.ActivationFunctionType.Gelu

    nc = bacc.Bacc("TRN2", target_bir_lowering=False, debug=False,
                   num_devices=E)

    xT = nc.dram_tensor("xT", [NK, 128, cap], bf16, kind="ExternalInput")
    # w1t[h][p][k][m] = w1[k*128+p, h*128+m]
    w1 = nc.dram_tensor("w1", [NH, 128, NK * 128], bf16, kind="ExternalInput")
    # w2 pre-scaled by W2S; [d][p][hk][m] = W2S*w2[hk*128+p, d*128+m]
    w28 = nc.dram_tensor("w28", [ND, 128, NH8 * 128], f8,
                         kind="ExternalInput")
    w2b = nc.dram_tensor("w2b", [ND, 128, (NH - NH8) * 128], bf16,
                         kind="ExternalInput")
    yT = nc.dram_tensor("yT", [ND, 128, cap], f32, kind="ExternalOutput")

    blocks = []
    t0 = 0
    for tb in sizes:
        blocks.append((t0, tb))
        t0 += tb

    with tile.TileContext(nc) as tc:
        with (
            tc.tile_pool(name="w1p", bufs=1) as w1p,
            tc.tile_pool(name="w2p", bufs=1) as w2p,
            tc.tile_pool(name="xp", bufs=2) as xp,
            tc.tile_pool(name="hp", bufs=1) as hp,
            tc.tile_pool(name="yp", bufs=4) as ypool,
            tc.tile_pool(name="ps1", bufs=4, space="PSUM") as ps1,
            tc.tile_pool(name="ps2", bufs=4, space="PSUM") as ps2,
        ):
            # resident weights
            w1sb = [w1p.tile([128, NK, 128], bf16, name=f"w1_{h}")
                    for h in range(NH)]
            w28sb = [w2p.tile([128, NH8, 128], f8, name=f"w28_{d}")
                     for d in range(ND)]
            w2bsb = [w2p.tile([128, NH - NH8, 128], bf16, name=f"w2b_{d}")
                     for d in range(ND)]

            def load_x(g, t0, tb):
                xt = [xp.tile([128, TB], bf16, name=f"x_{g}_{k}",
                              tag=f"x_{k}") for k in range(NK)]
                for k in range(NK):
                    nc.sync.dma_start(xt[k][:, :tb], xT.ap()[k][:, t0:t0 + tb])
                return xt

            # head: first GEMM1 chunk is gated by w1[0] + x block 0 only
            nc.sync.dma_start(w1sb[0][:], w1.ap()[0])
            xts = load_x(0, *blocks[0])
            for h in range(1, NH):
                nc.sync.dma_start(w1sb[h][:], w1.ap()[h])

            for g, (t0, tb) in enumerate(blocks):
                xt = xts
                if g + 1 < len(blocks):
                    xts = load_x(g + 1, *blocks[g + 1])

                # GEMM1 + GELU: h[:, n, :] = gelu(w1[n].T @ x)
                ht8 = hp.tile([128, NH8, TB], f8, name=f"h8_{g}", tag="h8")
                htb = hp.tile([128, NH - NH8, TB], bf16, name=f"hb_{g}",
                              tag="hb")
                for n in range(NH):
                    acc = ps1.tile([128, tb], f32, name=f"ps1_{g}_{n}",
                                   tag="ps1")
                    for k in range(NK):
                        nc.tensor.matmul(
                            acc[:, :tb],
                            w1sb[n][:, k, :],
                            xt[k][:, :tb],
                            start=(k == 0),
                            stop=(k == NK - 1),
                        )
                    dst = (ht8[:, n, :tb] if n < NH8
                           else htb[:, n - NH8, :tb])
                    nc.scalar.activation(dst, acc[:, :tb], GELU)

                if g == 0:
                    # w2 is first needed ~55us in; emitting its loads after
                    # GEMM1 keeps them out of the critical head DMA window
                    for d in range(ND):
                        nc.sync.dma_start(w28sb[d][:], w28.ap()[d])
                        nc.sync.dma_start(w2bsb[d][:], w2b.ap()[d])

                # GEMM2: y[d] = (w28[d].T @ h8 + w2b[d].T @ hb) / W2S
                for d in range(ND):
                    acc2 = ps2.tile([128, tb], f32, name=f"ps2_{g}_{d}",
                                    tag="ps2")
                    for p in range(NH8 // 2):
                        nc.tensor.matmul(
                            acc2[:, :tb],
                            w28sb[d][:, 2 * p:2 * p + 2, :],
                            ht8[:, 2 * p:2 * p + 2, :tb],
                            start=(p == 0),
                            stop=False,
                            perf_mode=DR,
                        )
                    for hk in range(NH - NH8):
                        nc.tensor.matmul(
                            acc2[:, :tb],
                            w2bsb[d][:, hk, :],
                            htb[:, hk, :tb],
                            start=False,
                            stop=(hk == NH - NH8 - 1),
                        )
                    yt = ypool.tile([128, TB], f32, name=f"y_{g}_{d}",
                                    tag="y")
                    nc.vector.tensor_scalar_mul(yt[:, :tb], acc2[:, :tb],
                                                1.0 / W2S)
                    # split the store so the tail drains across two queues
                    h1 = (tb // 8) * 4
                    nc.sync.dma_start(yT.ap()[d][:, t0:t0 + h1],
                                      yt[:, :h1])
                    nc.sync.dma_start(yT.ap()[d][:, t0 + h1:t0 + tb],
                                      yt[:, h1:tb])

    nc.compile()
    _BUILD_CACHE[cap] = (nc, cap)
    return nc, cap


def _route(x, gate_w):
    """Mirror the reference router with the exact same jax calls on the
    process-default backend, so the (discrete) top-k decisions match the
    reference bit-for-bit. Falls back to CPU if the default backend fails."""
    import jax
    import jax.numpy as jnp

    def run():
        logits = jnp.einsum("btd,de->bte", jnp.asarray(x),
                            jnp.asarray(gate_w))
        scores, indices = jax.lax.top_k(logits, 2)
        gates = jax.nn.softmax(scores, axis=-1)
        return (np.asarray(indices).reshape(-1, 2),
                np.asarray(gates, dtype=np.float32).reshape(-1, 2))

    try:
        return run()
    except Exception:
        with jax.default_device(jax.devices("cpu")[0]):
            return run()


def kernel(x, gate_w, w1, w2):
    import ml_dtypes
    from concourse.bass_utils import run_bass_kernel_spmd

    bf16 = ml_dtypes.bfloat16
    f8 = ml_dtypes.float8_e4m3

    x = np.asarray(x, dtype=np.float32)
    gate_w = np.asarray(gate_w, dtype=np.float32)
    w1 = np.asarray(w1, dtype=np.float32)
    w2 = np.asarray(w2, dtype=np.float32)

    B, T, _ = x.shape
    xf = x.reshape(-1, D)
    ntok = xf.shape[0]

    indices, gates = _route(x, gate_w)

    rows = []
    coefs = []
    for e in range(E):
        sel0 = indices[:, 0] == e
        sel1 = indices[:, 1] == e
        r = np.nonzero(sel0 | sel1)[0]
        c = np.where(sel0[r], gates[r, 0], gates[r, 1])
        rows.append(r)
        coefs.append(c.astype(np.float32))

    max_cnt = max(len(r) for r in rows)
    nc, cap = _build(max_cnt)

    in_maps = []
    for e in range(E):
        r = rows[e]
        xe = np.zeros((D, cap), dtype=np.float32)
        xe[:, :len(r)] = xf[r].T
        w2s = w2[e] * W2S
        w28 = (w2s[:NH8 * 128].reshape(NH8, 128, ND, 128)
               .transpose(2, 1, 0, 3).reshape(ND, 128, NH8 * 128))
        w2b = (w2s[NH8 * 128:].reshape(NH - NH8, 128, ND, 128)
               .transpose(2, 1, 0, 3).reshape(ND, 128, (NH - NH8) * 128))
        in_maps.append({
            "xT": np.ascontiguousarray(
                xe.reshape(NK, 128, cap).astype(bf16)),
            "w1": np.ascontiguousarray(
                w1[e].reshape(NK, 128, NH, 128).transpose(2, 1, 0, 3)
                .reshape(NH, 128, NK * 128).astype(bf16)),
            "w28": np.ascontiguousarray(
                np.clip(w28, -240, 240).astype(f8)),
            "w2b": np.ascontiguousarray(w2b.astype(bf16)),
        })

    res = run_bass_kernel_spmd(nc, in_maps, core_ids=list(range(E)),
                               trace=_TRACE)
    global _LAST_RES
    _LAST_RES = res

    out = np.zeros((ntok, D), dtype=np.float32)
    for e in range(E):
        r = rows[e]
        ye = res.results[e]["yT"].reshape(D, cap)
        out[r] += coefs[e][:, None] * ye[:, :len(r)].T
    return out.reshape(B, T, D)


# revision 4
# speedup vs baseline: 1.1917x; 1.0202x over previous
"""MoE layer (E=8 experts, top-2 routing, D=1024, hidden 4096, GELU) on 8
Trainium2 NeuronCores.

Strategy: expert parallelism, one expert per core. The router (gate matmul +
top-k + softmax) runs on the host with the exact jax calls of the reference,
tokens are gathered per expert and dispatched to one core each. Each core
runs the expert MLP
  y = gelu(x @ w1[e]) @ w2[e]
for its (capacity-padded) token set, mostly in bf16 (PE runs bf16 at the
same 1 col/cycle as fp32r, but FWL halves the stationary-weight load so
narrow remainder blocks aren't LDWEIGHTS-bound, and weights shrink 2x).
The first NH8=8 of GEMM2's 32 contraction chunks run as fp8e4 DoubleRow
matmuls (2 MACs/cell/cycle), which trims ~12% off GEMM2; the fp8
quantization error on a quarter of the contraction keeps the end-to-end
max relative error at ~1.7e-2 (vs 2e-2 budget; all-bf16 is 3.7e-3).
w2 is pre-scaled by 1024 (so its fp8 chunks stay out of the subnormal
range) and the output copy divides it back out.

Both weight matrices live fully resident in SBUF, so there is no per-pass
weight streaming and no DRAM scratch round-trip for partial outputs: per
512-token block, GEMM1 produces the full 4096-wide hidden in SBUF and GEMM2
immediately contracts it back down, writing the final f32 output once.
Blocks are sized in [256, 512] so every matmul's moving dim is >=256.
The host applies the gate coefficients and scatter-adds per token.
"""

import numpy as np

D = 1024        # token dim
E = 8           # experts == cores
HH = 4096       # hidden width (2*H)
NK = D // 128   # contraction chunks for GEMM1 (8)
NH = HH // 128  # hidden chunks (32)
NH8 = 10        # leading GEMM2 chunks computed in fp8e4 DoubleRow
W2S = 1024.0    # w2 pre-scale (divided out in the output copy)
ND = D // 128   # output chunks (8)
TB = 512        # max token block (psum bank width in fp32)

_BUILD_CACHE = {}
_TRACE = False      # test-only: capture an NTFF profile of the run
_LAST_RES = None    # test-only: last BassKernelResults


def _block_sizes(cap):
    """Split cap into blocks of width in [256, 512] (multiples of 4) so no
    matmul is LDWEIGHTS/overhead-bound. Widths 512 preferred."""
    cap = max(256, -(-cap // 4) * 4)
    nfull, rem = divmod(cap, TB)
    if rem == 0:
        return [TB] * nfull
    if rem >= 256 or nfull == 0:
        return [TB] * nfull + [max(256, rem)]
    # split the remainder + one full block into two blocks in [256, 512]
    tot = TB + rem
    a = -(-tot // 8) * 4
    return [TB] * (nfull - 1) + [a, tot - a]


def _build(cap):
    """Build + compile the per-core Bass program for capacity `cap`."""
    sizes = _block_sizes(cap)
    cap = sum(sizes)
    if cap in _BUILD_CACHE:
        return _BUILD_CACHE[cap]

    import concourse.mybir as mybir
    import concourse.tile as tile
    from concourse import bacc

    f32 = mybir.dt.float32
    bf16 = mybir.dt.bfloat16
    f8 = mybir.dt.float8e4
    DR = mybir.MatmulPerfMode.DoubleRow
    GELU = mybir.ActivationFunctionType.Gelu

    nc = bacc.Bacc("TRN2", target_bir_lowering=False, debug=False,
                   num_devices=E)

    xT = nc.dram_tensor("xT", [NK, 128, cap], bf16, kind="ExternalInput")
    # w1t[h][p][k][m] = w1[k*128+p, h*128+m]
    w1 = nc.dram_tensor("w1", [NH, 128, NK * 128], bf16, kind="ExternalInput")
    # w2 pre-scaled by W2S; [d][p][hk][m] = W2S*w2[hk*128+p, d*128+m]
    w28 = nc.dram_tensor("w28", [ND, 128, NH8 * 128], f8,
                         kind="ExternalInput")
    w2b = nc.dram_tensor("w2b", [ND, 128, (NH - NH8) * 128], bf16,
                         kind="ExternalInput")
    yT = nc.dram_tensor("yT", [ND, 128, cap], f32, kind="ExternalOutput")

    blocks = []
    t0 = 0
    for tb in sizes:
        blocks.append((t0, tb))
        t0 += tb

    with tile.TileContext(nc) as tc:
        with (
            tc.tile_pool(name="w1p", bufs=1) as w1p,
            tc.tile_pool(name="w2p", bufs=1) as w2p,
            tc.tile_pool(name="xp", bufs=2) as xp,
            tc.tile_pool(name="hp", bufs=1) as hp,
            tc.tile_pool(name="yp", bufs=4) as ypool,
            tc.tile_pool(name="ps1", bufs=4, space="PSUM") as ps1,
            tc.tile_pool(name="ps2", bufs=4, space="PSUM") as ps2,
        ):
            # resident weights
            w1sb = [w1p.tile([128, NK, 128], bf16, name=f"w1_{h}")
                    for h in range(NH)]
            w28sb = [w2p.tile([128, NH8, 128], f8, name=f"w28_{d}")
                     for d in range(ND)]
            w2bsb = [w2p.tile([128, NH - NH8, 128], bf16, name=f"w2b_{d}")
                     for d in range(ND)]

            def load_x(g, t0, tb):
                xt = [xp.tile([128, TB], bf16, name=f"x_{g}_{k}",
                              tag=f"x_{k}") for k in range(NK)]
                for k in range(NK):
                    nc.sync.dma_start(xt[k][:, :tb], xT.ap()[k][:, t0:t0 + tb])
                return xt

            # head: first GEMM1 chunk is gated by w1[0] + x block 0 only
            nc.sync.dma_start(w1sb[0][:], w1.ap()[0])
            xts = load_x(0, *blocks[0])
            for h in range(1, NH):
                nc.sync.dma_start(w1sb[h][:], w1.ap()[h])

            for g, (t0, tb) in enumerate(blocks):
                xt = xts
                if g + 1 < len(blocks):
                    xts = load_x(g + 1, *blocks[g + 1])

                # GEMM1 + GELU: h[:, n, :] = gelu(w1[n].T @ x)
                ht8 = hp.tile([128, NH8, TB], f8, name=f"h8_{g}", tag="h8")
                htb = hp.tile([128, NH - NH8, TB], bf16, name=f"hb_{g}",
                              tag="hb")
                for n in range(NH):
                    acc = ps1.tile([128, tb], f32, name=f"ps1_{g}_{n}",
                                   tag="ps1")
                    for k in range(NK):
                        nc.tensor.matmul(
                            acc[:, :tb],
                            w1sb[n][:, k, :],
                            xt[k][:, :tb],
                            start=(k == 0),
                            stop=(k == NK - 1),
                        )
                    dst = (ht8[:, n, :tb] if n < NH8
                           else htb[:, n - NH8, :tb])
                    nc.scalar.activation(dst, acc[:, :tb], GELU)

                if g == 0:
                    # w2 is first needed ~55us in; emitting its loads after
                    # GEMM1 keeps them out of the critical head DMA window
                    for d in range(ND):
                        nc.sync.dma_start(w28sb[d][:], w28.ap()[d])
                        nc.sync.dma_start(w2bsb[d][:], w2b.ap()[d])

                # GEMM2: y[d] = (w28[d].T @ h8 + w2b[d].T @ hb) / W2S
                for d in range(ND):
                    acc2 = ps2.tile([128, tb], f32, name=f"ps2_{g}_{d}",
                                    tag="ps2")
                    for p in range(NH8 // 2):
                        nc.tensor.matmul(
                            acc2[:, :tb],
                            w28sb[d][:, 2 * p:2 * p + 2, :],
                            ht8[:, 2 * p:2 * p + 2, :tb],
                            start=(p == 0),
                            stop=False,
                            perf_mode=DR,
                        )
                    for hk in range(NH - NH8):
                        nc.tensor.matmul(
                            acc2[:, :tb],
                            w2bsb[d][:, hk, :],
                            htb[:, hk, :tb],
                            start=False,
                            stop=(hk == NH - NH8 - 1),
                        )
                    yt = ypool.tile([128, TB], f32, name=f"y_{g}_{d}",
                                    tag="y")
                    nc.vector.tensor_scalar_mul(yt[:, :tb], acc2[:, :tb],
                                                1.0 / W2S)
                    # split the store so the tail drains across two queues
                    h1 = (tb // 8) * 4
                    nc.sync.dma_start(yT.ap()[d][:, t0:t0 + h1],
                                      yt[:, :h1])
                    nc.sync.dma_start(yT.ap()[d][:, t0 + h1:t0 + tb],
                                      yt[:, h1:tb])

    nc.compile()
    _BUILD_CACHE[cap] = (nc, cap)
    return nc, cap


def _route(x, gate_w):
    """Mirror the reference router with the exact same jax calls on the
    process-default backend, so the (discrete) top-k decisions match the
    reference bit-for-bit. Falls back to CPU if the default backend fails."""
    import jax
    import jax.numpy as jnp

    def run():
        logits = jnp.einsum("btd,de->bte", jnp.asarray(x),
                            jnp.asarray(gate_w))
        scores, indices = jax.lax.top_k(logits, 2)
        gates = jax.nn.softmax(scores, axis=-1)
        return (np.asarray(indices).reshape(-1, 2),
                np.asarray(gates, dtype=np.float32).reshape(-1, 2))

    try:
        return run()
    except Exception:
        with jax.default_device(jax.devices("cpu")[0]):
            return run()


def kernel(x, gate_w, w1, w2):
    import ml_dtypes
    from concourse.bass_utils import run_bass_kernel_spmd

    bf16 = ml_dtypes.bfloat16
    f8 = ml_dtypes.float8_e4m3

    x = np.asarray(x, dtype=np.float32)
    gate_w = np.asarray(gate_w, dtype=np.float32)
    w1 = np.asarray(w1, dtype=np.float32)
    w2 = np.asarray(w2, dtype=np.float32)

    B, T, _ = x.shape
    xf = x.reshape(-1, D)
    ntok = xf.shape[0]

    indices, gates = _route(x, gate_w)

    rows = []
    coefs = []
    for e in range(E):
        sel0 = indices[:, 0] == e
        sel1 = indices[:, 1] == e
        r = np.nonzero(sel0 | sel1)[0]
        c = np.where(sel0[r], gates[r, 0], gates[r, 1])
        rows.append(r)
        coefs.append(c.astype(np.float32))

    max_cnt = max(len(r) for r in rows)
    nc, cap = _build(max_cnt)

    in_maps = []
    for e in range(E):
        r = rows[e]
        xe = np.zeros((D, cap), dtype=np.float32)
        xe[:, :len(r)] = xf[r].T
        w2s = w2[e] * W2S
        w28 = (w2s[:NH8 * 128].reshape(NH8, 128, ND, 128)
               .transpose(2, 1, 0, 3).reshape(ND, 128, NH8 * 128))
        w2b = (w2s[NH8 * 128:].reshape(NH - NH8, 128, ND, 128)
               .transpose(2, 1, 0, 3).reshape(ND, 128, (NH - NH8) * 128))
        in_maps.append({
            "xT": np.ascontiguousarray(
                xe.reshape(NK, 128, cap).astype(bf16)),
            "w1": np.ascontiguousarray(
                w1[e].reshape(NK, 128, NH, 128).transpose(2, 1, 0, 3)
                .reshape(NH, 128, NK * 128).astype(bf16)),
            "w28": np.ascontiguousarray(
                np.clip(w28, -240, 240).astype(f8)),
            "w2b": np.ascontiguousarray(w2b.astype(bf16)),
        })

    res = run_bass_kernel_spmd(nc, in_maps, core_ids=list(range(E)),
                               trace=_TRACE)
    global _LAST_RES
    _LAST_RES = res

    out = np.zeros((ntok, D), dtype=np.float32)
    for e in range(E):
        r = rows[e]
        ye = res.results[e]["yT"].reshape(D, cap)
        out[r] += coefs[e][:, None] * ye[:, :len(r)].T
    return out.reshape(B, T, D)
